# revision 1
# baseline (speedup 1.0000x reference)
"""Bag-of-words embedding + sum-pool + FC kernel for Trainium2, 8 NeuronCores.

Reference computation (all f32):
    mask    = (seq != 0)
    pooled  = sum_l emb_table[seq[b,l]] * mask[b,l]        # [B, E]
    out     = pooled @ fc_w.T + fc_b                       # [B, C]

Strategy (data-parallel over batch, one SPMD NEFF on 8 cores):
  Host prep:
    - Zero row 0 of the table (mask becomes free: pad tokens gather zeros).
    - Tokens are gathered with dma_gather (int16 indices), so the vocab is
      split into 4 windows of 32000 rows; a rebuilt table ("table2",
      4 x 32768 rows) carries 768 all-zero pad rows per window.  Per-row
      window segments are padded to a per-group slot count; pad indices are
      spread across the 768 zero rows to avoid a hot HBM row.
    - Rows are bin-packed: all B rows sorted by window-count profile,
      consecutive 128-row blocks get near-identical slot needs; block g*8+k
      goes to core k group g.  Outputs are inverse-permuted on the host.
    - Each row's tokens are sorted (ascending per window) - the per-SDMA
      address streams become ascending, which measures ~2x faster.
    - Index tensors are built host-side in dma_gather's wrapped layout
      (position i = idx[i%16, i//16], replicated to 128 partitions), ordered
      column-major so batch row (g*128+p) lands entirely in partition p.
  Device, per 128-row group:
    - dma_gather: 8-slot (1024-index) instructions, queue_num rotating over
      4 SWDGE queues (4 Q7 pairs generate descriptors in parallel), windows
      round-robined so concurrent descriptors spread across HBM.
    - VectorE: strided tensor_reduce over each 24-slot tile -> pooled
      [rows=128, E] accumulator.
    - TensorE: transpose pooled -> [E, rows]; FC matmul against host-side
      pre-transposed fc_w; bias add fused with PSUM evacuation on VectorE.
"""

import os as _os

import numpy as np

# Problem shape (hardcoded per contract).
B, L, V, E, C = 16384, 200, 100000, 128, 1000
NCORES = 8
P = 128

# int16-addressable vocab windows
CHUNK = 32000  # usable rows per window
CW = 32768  # window stride in table2 (rows >= window size are zero pads)
NCH = (V + CHUNK - 1) // CHUNK  # 4
NPADROWS = 768  # spread pad targets per window
GSLOT = int(_os.environ.get("BOW_GSLOT", "8"))  # slots per dma_gather instr
TSLOT = int(_os.environ.get("BOW_TSLOT", "24"))  # slots per reduce tile
NQ = int(_os.environ.get("BOW_NQ", "4"))  # SWDGE queues
GBUFS = int(_os.environ.get("BOW_GBUFS", "6"))  # gather tile pool bufs

_WBOUNDS = [min(c * CHUNK, V) for c in range(NCH + 1)]
_WSIZE = [_WBOUNDS[c + 1] - _WBOUNDS[c] for c in range(NCH)]


def _build_table2(table):
    table2 = np.zeros((NCH * CW, E), dtype=np.float32)
    for c in range(NCH):
        lo, hi = _WBOUNDS[c], _WBOUNDS[c + 1]
        table2[c * CW : c * CW + (hi - lo)] = table[lo:hi]
    table2[0] = 0.0  # token id 0 is padding
    return table2


def _chunk_counts(sseq):
    """sseq: [N, L] sorted rows -> counts [N, NCH] per vocab window."""
    b = np.asarray(_WBOUNDS)
    N = sseq.shape[0]
    pos = np.empty((N, NCH + 1), dtype=np.int64)
    for c in range(NCH + 1):
        pos[:, c] = np.sum(sseq < b[c], axis=1)
    return pos[:, 1:] - pos[:, :-1], pos[:, :-1]


def _pack_rows(sseq):
    """Sort rows by window-count profile so 128-row blocks have tight maxes.

    Returns perm (block-ordered original row indices) and the per-block
    counts [NBLK, NCH]."""
    cnt, _ = _chunk_counts(sseq)
    # quantized first key packs blocks tighter across all windows jointly
    perm = np.lexsort((cnt[:, 2], cnt[:, 1], cnt[:, 0] // 8))
    return perm, cnt[perm]


def _build_locals_blocks(sseq_perm, S_per_group):
    """sseq_perm: [B, L] sorted rows in block order.  S_per_group: [G, NCH].
    Returns A: [B, S_total_max...] -> per-block slot arrays as a list of
    [128, S_total_g] int16 arrays (block b uses group g = b // NCORES)."""
    NBLK = sseq_perm.shape[0] // P
    out = []
    for blk in range(NBLK):
        g = blk // NCORES
        S = S_per_group[g]
        offs = np.concatenate([[0], np.cumsum(S)])
        Ablk = np.empty((P, offs[-1]), dtype=np.int16)
        rows = sseq_perm[blk * P : (blk + 1) * P]
        for c in range(NCH):
            lo, hi = _WBOUNDS[c], _WBOUNDS[c + 1]
            m = (rows >= lo) & (rows < hi)
            j = np.cumsum(m, axis=1) - 1
            # spread pad targets over the window's zero rows
            pslot = np.arange(S[c])[None, :]
            prow = np.arange(P)[:, None]
            padbase = _WSIZE[c]
            padspan = min(NPADROWS, CW - padbase)
            block = (padbase + (prow * 37 + pslot * 13) % padspan).astype(np.int16)
            rr, cc = np.nonzero(m)
            block[rr, j[rr, cc]] = (rows[rr, cc] - lo).astype(np.int16)
            Ablk[:, offs[c] : offs[c + 1]] = block
        out.append(Ablk)
    return out


def _wrap_idxs(a):
    """a: [P, slots] per-partition local idxs -> dma_gather wrapped layout
    [128, 8*slots] int16.  Position i = (partition i%128, slot i//128);
    wrapped[p16, col] = flat[col*16 + p16], replicated to 128 partitions."""
    flat = a.T.ravel()  # position s*128+p = a[p, s]
    w16 = flat.reshape(-1, 16).T  # [16, 8*slots]
    return np.tile(w16, (8, 1)).astype(np.int16)


def _group_plan(S):
    """S: [NCH] slots per window -> interleaved tile plan.

    Returns a list of (c, tile_slot_off, tile_nslots) in window-round-robin
    order; each tile gets GSLOT-sized gathers and one reduce."""
    tiles_per_c = []
    for c in range(NCH):
        t = []
        s = 0
        while s < S[c]:
            n = min(TSLOT, int(S[c]) - s)
            t.append((c, s, n))
            s += n
        tiles_per_c.append(t)
    plan = []
    j = 0
    while any(j < len(t) for t in tiles_per_c):
        for c in range(NCH):
            if j < len(tiles_per_c[c]):
                plan.append(tiles_per_c[c][j])
        j += 1
    return plan


def _build_bass(S_per_group, G, reps=1, stages="full"):
    import concourse.bacc as bacc
    import concourse.tile as tile
    from concourse import mybir
    from concourse.masks import make_identity

    S_per_group = [[int(s) for s in S] for S in S_per_group]
    S_tot = [sum(S) for S in S_per_group]
    g_off = np.concatenate([[0], np.cumsum(S_tot)])  # idx col offsets (slots)
    total_slots = int(g_off[-1])

    nc = bacc.Bacc("TRN2", target_bir_lowering=False, num_swdge_queues=NQ)
    f32 = mybir.dt.float32
    _qctr = [0]

    def _next_q():
        q = _qctr[0] % NQ
        _qctr[0] += 1
        return q

    table2_d = nc.dram_tensor("table2", [NCH * CW, E], f32, kind="ExternalInput")
    idx_d = nc.dram_tensor("idx", [P, 8 * total_slots], mybir.dt.int16,
                           kind="ExternalInput")
    wt_d = nc.dram_tensor("wt", [E, C], f32, kind="ExternalInput")
    bias_d = nc.dram_tensor("bias", [P, C], f32, kind="ExternalInput")
    out_d = nc.dram_tensor("out", [G * P, C], f32, kind="ExternalOutput")

    with tile.TileContext(nc) as tc:
        with (
            tc.tile_pool(name="const", bufs=1) as constp,
            tc.tile_pool(name="idxp", bufs=2) as idxp,
            tc.tile_pool(name="gat", bufs=GBUFS) as gatp,
            tc.tile_pool(name="red", bufs=3) as redp,
            tc.tile_pool(name="acc", bufs=2) as accp,
            tc.tile_pool(name="pt", bufs=2) as ptp,
            tc.tile_pool(name="outp", bufs=2) as outp,
            tc.tile_pool(name="ps_t", bufs=2, space="PSUM") as ps_t,
            tc.tile_pool(name="ps_fc", bufs=2, space="PSUM") as ps_fc,
        ):
            ident = constp.tile([P, P], f32)
            make_identity(nc, ident[:])
            wt_sb = constp.tile([E, C], f32)
            nc.sync.dma_start(out=wt_sb[:], in_=wt_d[:])
            bias_sb = constp.tile([P, C], f32)
            nc.sync.dma_start(out=bias_sb[:], in_=bias_d[:])

            for g in [g for _ in range(reps) for g in range(G)]:
                S = S_per_group[g]
                st = S_tot[g]
                off0 = int(g_off[g])
                w_off = np.concatenate([[0], np.cumsum(S)])

                idx_sb = idxp.tile([P, 8 * st], mybir.dt.int16, tag="idx")
                nc.sync.dma_start(
                    out=idx_sb[:],
                    in_=idx_d[:, 8 * off0 : 8 * (off0 + st)],
                )

                acc = accp.tile([P, E], f32)
                first = True
                for c, toff, n in _group_plan(S):
                    gt = gatp.tile([P, TSLOT, E], f32, tag="gat")
                    col = int(w_off[c]) + toff  # slot col within group idx
                    for j in range(0, n, GSLOT):
                        m = min(GSLOT, n - j)
                        nc.gpsimd.dma_gather(
                            gt[:, j : j + m, :],
                            table2_d[c * CW : (c + 1) * CW, :],
                            idx_sb[:, 8 * (col + j) : 8 * (col + j + m)],
                            128 * m,
                            128 * m,
                            E,
                            single_packet=False,
                            queue_num=_next_q(),
                        )
                    if stages == "gatheronly":
                        if first:
                            nc.vector.tensor_copy(out=acc[:], in_=gt[:, 0, :])
                            first = False
                        continue
                    if first:
                        nc.vector.tensor_reduce(
                            out=acc[:],
                            in_=gt[:, :n, :].rearrange("p s e -> p e s"),
                            axis=mybir.AxisListType.X,
                            op=mybir.AluOpType.add,
                        )
                        first = False
                    else:
                        r = redp.tile([P, E], f32)
                        nc.vector.tensor_reduce(
                            out=r[:],
                            in_=gt[:, :n, :].rearrange("p s e -> p e s"),
                            axis=mybir.AxisListType.X,
                            op=mybir.AluOpType.add,
                        )
                        nc.vector.tensor_tensor(
                            out=acc[:], in0=acc[:], in1=r[:],
                            op=mybir.AluOpType.add,
                        )

                if stages in ("gatheronly", "nofc"):
                    out_sb = outp.tile([P, C], f32)
                    nc.vector.tensor_copy(out=out_sb[:, :E], in_=acc[:])
                    nc.sync.dma_start(
                        out=out_d[g * P : (g + 1) * P, :E], in_=out_sb[:, :E]
                    )
                    continue

                # pooled [rows, E] -> pooledT [E, rows]
                psT = ps_t.tile([E, P], f32)
                nc.tensor.transpose(out=psT[:], in_=acc[:], identity=ident[:])
                pooledT = ptp.tile([E, P], f32)
                nc.scalar.copy(out=pooledT[:], in_=psT[:])

                out_sb = outp.tile([P, C], f32)
                for n0 in range(0, C, 512):
                    n1 = min(n0 + 512, C)
                    ps = ps_fc.tile([P, 512], f32, tag="psfc")
                    nc.tensor.matmul(
                        out=ps[:, : n1 - n0],
                        lhsT=pooledT[:],
                        rhs=wt_sb[:, n0:n1],
                        start=True,
                        stop=True,
                    )
                    nc.vector.tensor_tensor(
                        out=out_sb[:, n0:n1],
                        in0=ps[:, : n1 - n0],
                        in1=bias_sb[:, n0:n1],
                        op=mybir.AluOpType.add,
                    )
                nc.sync.dma_start(out=out_d[g * P : (g + 1) * P, :], in_=out_sb[:])

    nc.compile()
    return nc


_CACHE = {}


def _get_nc(S_per_group, G, reps=1, stages="full"):
    key = (tuple(tuple(int(s) for s in S) for S in S_per_group), G, reps, stages)
    if key not in _CACHE:
        _CACHE[key] = _build_bass(S_per_group, G, reps, stages)
    return _CACHE[key]


def _host_prep(seq, emb_table, fc_w, fc_b):
    """Shared host-side prep.  Returns (in_maps, S_per_group, G, perm)."""
    seq64 = np.asarray(seq).astype(np.int64)
    table = np.asarray(emb_table, dtype=np.float32)
    fc_w = np.asarray(fc_w, dtype=np.float32)
    fc_b = np.asarray(fc_b, dtype=np.float32)

    Btot, Ltot = seq64.shape
    assert (Btot, Ltot) == (B, L) and table.shape == (V, E), (seq64.shape, table.shape)
    BC = Btot // NCORES
    G = BC // P

    table2 = _build_table2(table)
    wt = np.ascontiguousarray(fc_w.T)  # [E, C]
    bias = np.broadcast_to(fc_b, (P, C)).copy()

    sseq = np.sort(seq64, axis=1)
    perm, cnt_perm = _pack_rows(sseq)  # global block packing
    sseq_perm = sseq[perm]

    NBLK = Btot // P
    blk_max = cnt_perm.reshape(NBLK, P, NCH).max(axis=1)  # [NBLK, NCH]
    # group g serves blocks g*NCORES..g*NCORES+7 (core k <- block g*8+k)
    S_per_group = blk_max.reshape(G, NCORES, NCH).max(axis=1)  # [G, NCH]

    A_blocks = _build_locals_blocks(sseq_perm, S_per_group)

    # per-core flat idx tensors [P, 8*total_slots]
    S_tot = S_per_group.sum(axis=1)
    total_slots = int(S_tot.sum())
    in_maps = []
    for k in range(NCORES):
        idx_k = np.empty((P, 8 * total_slots), dtype=np.int16)
        colo = 0
        for g in range(G):
            blk = g * NCORES + k
            w = _wrap_idxs(A_blocks[blk])
            idx_k[:, colo : colo + w.shape[1]] = w
            colo += 8 * int(S_tot[g])
        in_maps.append({"table2": table2, "idx": idx_k, "wt": wt, "bias": bias})
    return in_maps, S_per_group, G, perm


def _assemble_out(results, perm, G):
    """results: list of per-core {'out': [G*P, C]} -> full [B, C] in original
    row order."""
    out = np.empty((B, C), dtype=np.float32)
    for k in range(NCORES):
        ok = results[k]["out"]
        for g in range(G):
            blk = g * NCORES + k
            out[perm[blk * P : (blk + 1) * P]] = ok[g * P : (g + 1) * P]
    return out


def kernel(seq, emb_table, fc_w, fc_b, _trace=False):
    from concourse.bass_utils import run_bass_kernel_spmd

    in_maps, S_per_group, G, perm = _host_prep(seq, emb_table, fc_w, fc_b)
    nc = _get_nc(S_per_group, G)
    res = run_bass_kernel_spmd(
        nc, in_maps, core_ids=list(range(NCORES)), trace=_trace
    )
    if _trace:
        kernel._last_perf = res
    return _assemble_out(res.results, perm, G)


kernel._last_perf = None



# revision 2
# speedup vs baseline: 21885.1922x; 21885.1922x over previous
"""Bag-of-words embedding + sum-pool + FC kernel for Trainium2, 8 NeuronCores.

Reference computation (all f32):
    mask    = (seq != 0)
    pooled  = sum_l emb_table[seq[b,l]] * mask[b,l]        # [B, E]
    out     = pooled @ fc_w.T + fc_b                       # [B, C]

The gather stream is DMA-descriptor-rate-bound (~1.3-2 ns per 256B row
descriptor across the 4 SWDGE queues), so the table is stored bf16 (halves
bytes at equal descriptor count; rel err ~1.6e-3 vs 2e-2 budget) and gather
instructions are large (24 slots x 128 rows = 3072 descriptors) with a
12-deep tile pool to keep all queues fed.

Strategy (data-parallel over batch, one SPMD NEFF on 8 cores):
  Host prep:
    - Zero row 0 of the table (mask becomes free: pad tokens gather zeros).
    - Tokens are gathered with dma_gather (int16 indices), so the vocab is
      split into 4 windows of 32000 rows; a rebuilt table ("table2",
      4 x 32768 rows) carries 768 all-zero pad rows per window.  Per-row
      window segments are padded to a per-group slot count; pad indices are
      spread across the 768 zero rows to avoid a hot HBM row.
    - Rows are bin-packed: all B rows sorted by window-count profile,
      consecutive 128-row blocks get near-identical slot needs; block g*8+k
      goes to core k group g.  Outputs are inverse-permuted on the host.
    - Each row's tokens are sorted (ascending per window) - the per-SDMA
      address streams become ascending, which measures ~2x faster.
    - Index tensors are built host-side in dma_gather's wrapped layout
      (position i = idx[i%16, i//16], replicated to 128 partitions), ordered
      column-major so batch row (g*128+p) lands entirely in partition p.
  Device, per 128-row group:
    - dma_gather: 8-slot (1024-index) instructions, queue_num rotating over
      4 SWDGE queues (4 Q7 pairs generate descriptors in parallel), windows
      round-robined so concurrent descriptors spread across HBM.
    - VectorE: strided tensor_reduce over each 24-slot tile -> pooled
      [rows=128, E] accumulator.
    - TensorE: transpose pooled -> [E, rows]; FC matmul against host-side
      pre-transposed fc_w; bias add fused with PSUM evacuation on VectorE.
"""

import os as _os

import ml_dtypes
import numpy as np

BF16 = ml_dtypes.bfloat16

# Problem shape (hardcoded per contract).
B, L, V, E, C = 16384, 200, 100000, 128, 1000
NCORES = 8
P = 128

# int16-addressable vocab windows
CHUNK = 32000  # usable rows per window
CW = 32768  # window stride in table2 (rows >= window size are zero pads)
NCH = (V + CHUNK - 1) // CHUNK  # 4
NPADROWS = 768  # spread pad targets per window
GSLOT = int(_os.environ.get("BOW_GSLOT", "24"))  # slots per dma_gather instr
TSLOT = int(_os.environ.get("BOW_TSLOT", "24"))  # slots per reduce tile
NQ = int(_os.environ.get("BOW_NQ", "4"))  # SWDGE queues
GBUFS = int(_os.environ.get("BOW_GBUFS", "12"))  # gather tile pool bufs

_WBOUNDS = [min(c * CHUNK, V) for c in range(NCH + 1)]
_WSIZE = [_WBOUNDS[c + 1] - _WBOUNDS[c] for c in range(NCH)]


def _build_table2(table):
    table2 = np.zeros((NCH * CW, E), dtype=BF16)
    for c in range(NCH):
        lo, hi = _WBOUNDS[c], _WBOUNDS[c + 1]
        table2[c * CW : c * CW + (hi - lo)] = table[lo:hi].astype(BF16)
    table2[0] = 0.0  # token id 0 is padding
    return table2


def _chunk_counts(sseq):
    """sseq: [N, L] sorted rows -> counts [N, NCH] per vocab window."""
    b = np.asarray(_WBOUNDS)
    N = sseq.shape[0]
    pos = np.empty((N, NCH + 1), dtype=np.int64)
    for c in range(NCH + 1):
        pos[:, c] = np.sum(sseq < b[c], axis=1)
    return pos[:, 1:] - pos[:, :-1], pos[:, :-1]


def _pack_rows(sseq):
    """Sort rows by window-count profile so 128-row blocks have tight maxes.

    Returns perm (block-ordered original row indices) and the per-block
    counts [NBLK, NCH]."""
    cnt, _ = _chunk_counts(sseq)
    # quantized first key packs blocks tighter across all windows jointly
    perm = np.lexsort((cnt[:, 2], cnt[:, 1], cnt[:, 0] // 8))
    return perm, cnt[perm]


def _build_locals_blocks(sseq_perm, S_per_group):
    """sseq_perm: [B, L] sorted rows in block order.  S_per_group: [G, NCH].
    Returns A: [B, S_total_max...] -> per-block slot arrays as a list of
    [128, S_total_g] int16 arrays (block b uses group g = b // NCORES)."""
    NBLK = sseq_perm.shape[0] // P
    out = []
    for blk in range(NBLK):
        g = blk // NCORES
        S = S_per_group[g]
        offs = np.concatenate([[0], np.cumsum(S)])
        Ablk = np.empty((P, offs[-1]), dtype=np.int16)
        rows = sseq_perm[blk * P : (blk + 1) * P]
        for c in range(NCH):
            lo, hi = _WBOUNDS[c], _WBOUNDS[c + 1]
            m = (rows >= lo) & (rows < hi)
            j = np.cumsum(m, axis=1) - 1
            # spread pad targets over the window's zero rows
            pslot = np.arange(S[c])[None, :]
            prow = np.arange(P)[:, None]
            padbase = _WSIZE[c]
            padspan = min(NPADROWS, CW - padbase)
            block = (padbase + (prow * 37 + pslot * 13) % padspan).astype(np.int16)
            rr, cc = np.nonzero(m)
            block[rr, j[rr, cc]] = (rows[rr, cc] - lo).astype(np.int16)
            Ablk[:, offs[c] : offs[c + 1]] = block
        out.append(Ablk)
    return out


def _wrap_idxs(a):
    """a: [P, slots] per-partition local idxs -> dma_gather wrapped layout
    [128, 8*slots] int16.  Position i = (partition i%128, slot i//128);
    wrapped[p16, col] = flat[col*16 + p16], replicated to 128 partitions."""
    flat = a.T.ravel()  # position s*128+p = a[p, s]
    w16 = flat.reshape(-1, 16).T  # [16, 8*slots]
    return np.tile(w16, (8, 1)).astype(np.int16)


def _group_plan(S):
    """S: [NCH] slots per window -> interleaved tile plan.

    Returns a list of (c, tile_slot_off, tile_nslots) in window-round-robin
    order; each tile gets GSLOT-sized gathers and one reduce."""
    tiles_per_c = []
    for c in range(NCH):
        t = []
        s = 0
        while s < S[c]:
            n = min(TSLOT, int(S[c]) - s)
            t.append((c, s, n))
            s += n
        tiles_per_c.append(t)
    plan = []
    j = 0
    while any(j < len(t) for t in tiles_per_c):
        for c in range(NCH):
            if j < len(tiles_per_c[c]):
                plan.append(tiles_per_c[c][j])
        j += 1
    return plan


def _build_bass(S_per_group, G, reps=1, stages="full"):
    import concourse.bacc as bacc
    import concourse.tile as tile
    from concourse import mybir
    from concourse.masks import make_identity

    S_per_group = [[int(s) for s in S] for S in S_per_group]
    S_tot = [sum(S) for S in S_per_group]
    g_off = np.concatenate([[0], np.cumsum(S_tot)])  # idx col offsets (slots)
    total_slots = int(g_off[-1])

    nc = bacc.Bacc("TRN2", target_bir_lowering=False, num_swdge_queues=NQ)
    f32 = mybir.dt.float32
    bf16 = mybir.dt.bfloat16
    _qctr = [0]

    def _next_q():
        q = _qctr[0] % NQ
        _qctr[0] += 1
        return q

    table2_d = nc.dram_tensor("table2", [NCH * CW, E], bf16, kind="ExternalInput")
    idx_d = nc.dram_tensor("idx", [P, 8 * total_slots], mybir.dt.int16,
                           kind="ExternalInput")
    wt_d = nc.dram_tensor("wt", [E, C], f32, kind="ExternalInput")
    bias_d = nc.dram_tensor("bias", [P, C], f32, kind="ExternalInput")
    out_d = nc.dram_tensor("out", [G * P, C], f32, kind="ExternalOutput")

    with tile.TileContext(nc) as tc:
        with (
            tc.tile_pool(name="const", bufs=1) as constp,
            tc.tile_pool(name="idxp", bufs=2) as idxp,
            tc.tile_pool(name="gat", bufs=GBUFS) as gatp,
            tc.tile_pool(name="red", bufs=3) as redp,
            tc.tile_pool(name="acc", bufs=2) as accp,
            tc.tile_pool(name="pt", bufs=2) as ptp,
            tc.tile_pool(name="outp", bufs=2) as outp,
            tc.tile_pool(name="ps_t", bufs=2, space="PSUM") as ps_t,
            tc.tile_pool(name="ps_fc", bufs=2, space="PSUM") as ps_fc,
        ):
            ident = constp.tile([P, P], f32)
            make_identity(nc, ident[:])
            wt_sb = constp.tile([E, C], f32)
            nc.sync.dma_start(out=wt_sb[:], in_=wt_d[:])
            bias_sb = constp.tile([P, C], f32)
            nc.sync.dma_start(out=bias_sb[:], in_=bias_d[:])

            for g in [g for _ in range(reps) for g in range(G)]:
                S = S_per_group[g]
                st = S_tot[g]
                off0 = int(g_off[g])
                w_off = np.concatenate([[0], np.cumsum(S)])

                idx_sb = idxp.tile([P, 8 * st], mybir.dt.int16, tag="idx")
                nc.sync.dma_start(
                    out=idx_sb[:],
                    in_=idx_d[:, 8 * off0 : 8 * (off0 + st)],
                )

                acc = accp.tile([P, E], f32)
                first = True
                for c, toff, n in _group_plan(S):
                    gt = gatp.tile([P, TSLOT, E], bf16, tag="gat")
                    col = int(w_off[c]) + toff  # slot col within group idx
                    for j in range(0, n, GSLOT):
                        m = min(GSLOT, n - j)
                        nc.gpsimd.dma_gather(
                            gt[:, j : j + m, :],
                            table2_d[c * CW : (c + 1) * CW, :],
                            idx_sb[:, 8 * (col + j) : 8 * (col + j + m)],
                            128 * m,
                            128 * m,
                            E,
                            single_packet=False,
                            queue_num=_next_q(),
                        )
                    if stages == "gatheronly":
                        if first:
                            nc.vector.tensor_copy(out=acc[:], in_=gt[:, 0, :])
                            first = False
                        continue
                    if first:
                        nc.vector.tensor_reduce(
                            out=acc[:],
                            in_=gt[:, :n, :].rearrange("p s e -> p e s"),
                            axis=mybir.AxisListType.X,
                            op=mybir.AluOpType.add,
                        )
                        first = False
                    else:
                        r = redp.tile([P, E], f32)
                        nc.vector.tensor_reduce(
                            out=r[:],
                            in_=gt[:, :n, :].rearrange("p s e -> p e s"),
                            axis=mybir.AxisListType.X,
                            op=mybir.AluOpType.add,
                        )
                        nc.vector.tensor_tensor(
                            out=acc[:], in0=acc[:], in1=r[:],
                            op=mybir.AluOpType.add,
                        )

                if stages in ("gatheronly", "nofc"):
                    out_sb = outp.tile([P, C], f32)
                    nc.vector.tensor_copy(out=out_sb[:, :E], in_=acc[:])
                    nc.sync.dma_start(
                        out=out_d[g * P : (g + 1) * P, :E], in_=out_sb[:, :E]
                    )
                    continue

                # pooled [rows, E] -> pooledT [E, rows]
                psT = ps_t.tile([E, P], f32)
                nc.tensor.transpose(out=psT[:], in_=acc[:], identity=ident[:])
                pooledT = ptp.tile([E, P], f32)
                nc.scalar.copy(out=pooledT[:], in_=psT[:])

                out_sb = outp.tile([P, C], f32)
                for n0 in range(0, C, 512):
                    n1 = min(n0 + 512, C)
                    ps = ps_fc.tile([P, 512], f32, tag="psfc")
                    nc.tensor.matmul(
                        out=ps[:, : n1 - n0],
                        lhsT=pooledT[:],
                        rhs=wt_sb[:, n0:n1],
                        start=True,
                        stop=True,
                    )
                    nc.vector.tensor_tensor(
                        out=out_sb[:, n0:n1],
                        in0=ps[:, : n1 - n0],
                        in1=bias_sb[:, n0:n1],
                        op=mybir.AluOpType.add,
                    )
                nc.sync.dma_start(out=out_d[g * P : (g + 1) * P, :], in_=out_sb[:])

    nc.compile()
    return nc


_CACHE = {}


def _get_nc(S_per_group, G, reps=1, stages="full"):
    key = (tuple(tuple(int(s) for s in S) for S in S_per_group), G, reps, stages)
    if key not in _CACHE:
        _CACHE[key] = _build_bass(S_per_group, G, reps, stages)
    return _CACHE[key]


def _host_prep(seq, emb_table, fc_w, fc_b):
    """Shared host-side prep.  Returns (in_maps, S_per_group, G, perm)."""
    seq64 = np.asarray(seq).astype(np.int64)
    table = np.asarray(emb_table, dtype=np.float32)
    fc_w = np.asarray(fc_w, dtype=np.float32)
    fc_b = np.asarray(fc_b, dtype=np.float32)

    Btot, Ltot = seq64.shape
    assert (Btot, Ltot) == (B, L) and table.shape == (V, E), (seq64.shape, table.shape)
    BC = Btot // NCORES
    G = BC // P

    table2 = _build_table2(table)
    wt = np.ascontiguousarray(fc_w.T)  # [E, C]
    bias = np.broadcast_to(fc_b, (P, C)).copy()

    sseq = np.sort(seq64, axis=1)
    perm, cnt_perm = _pack_rows(sseq)  # global block packing
    sseq_perm = sseq[perm]

    NBLK = Btot // P
    blk_max = cnt_perm.reshape(NBLK, P, NCH).max(axis=1)  # [NBLK, NCH]
    # group g serves blocks g*NCORES..g*NCORES+7 (core k <- block g*8+k)
    S_per_group = blk_max.reshape(G, NCORES, NCH).max(axis=1)  # [G, NCH]

    A_blocks = _build_locals_blocks(sseq_perm, S_per_group)

    # per-core flat idx tensors [P, 8*total_slots]
    S_tot = S_per_group.sum(axis=1)
    total_slots = int(S_tot.sum())
    in_maps = []
    for k in range(NCORES):
        idx_k = np.empty((P, 8 * total_slots), dtype=np.int16)
        colo = 0
        for g in range(G):
            blk = g * NCORES + k
            w = _wrap_idxs(A_blocks[blk])
            idx_k[:, colo : colo + w.shape[1]] = w
            colo += 8 * int(S_tot[g])
        in_maps.append({"table2": table2, "idx": idx_k, "wt": wt, "bias": bias})
    return in_maps, S_per_group, G, perm


def _assemble_out(results, perm, G):
    """results: list of per-core {'out': [G*P, C]} -> full [B, C] in original
    row order."""
    out = np.empty((B, C), dtype=np.float32)
    for k in range(NCORES):
        ok = results[k]["out"]
        for g in range(G):
            blk = g * NCORES + k
            out[perm[blk * P : (blk + 1) * P]] = ok[g * P : (g + 1) * P]
    return out


def kernel(seq, emb_table, fc_w, fc_b, _trace=False):
    from concourse.bass_utils import run_bass_kernel_spmd

    in_maps, S_per_group, G, perm = _host_prep(seq, emb_table, fc_w, fc_b)
    nc = _get_nc(S_per_group, G)
    res = run_bass_kernel_spmd(
        nc, in_maps, core_ids=list(range(NCORES)), trace=_trace
    )
    if _trace:
        kernel._last_perf = res
    return _assemble_out(res.results, perm, G)


kernel._last_perf = None

